# revision 1
# baseline (speedup 1.0000x reference)
"""Trainium2 Bass kernel for nn_LogReg_455266533602.

Math: out[b] = sum_t W[0, text[t, b]] + bias  (the [B,V] histogram times W
collapses to a gather-and-reduce; the histogram is never materialized).

Strategy (8 NeuronCores, data-parallel over the 8192-phrase batch):
  - Each core handles 1024 phrases (100 tokens each = 102400 lookups).
  - Vocab padded to 32768 = 16 segments x 2048. SBUF table [128, 2048]:
    partition p holds W[2048*(p%16) : 2048*(p%16+1)] (the 16-segment
    split tiled 8x across the 8 gpsimd core groups).
  - gpsimd ap_gather: for each 16-partition group, gather that group's
    12800 token offsets (o = v & 2047) from every partition's row. The
    wanted value for token j lands at partition 16g + (v >> 11).
  - A host-built one-hot mask [128, 12800] zeroes the 15 wrong lanes
    (DVE multiply), a segmented reduce sums each phrase's 100 tokens,
    and one small PE matmul folds the 16 partitions of each group.
"""
from contextlib import suppress

import numpy as np

import concourse.bacc as bacc
import concourse.mybir as mybir
import concourse.tile as tile
from concourse.bass_utils import run_bass_kernel_spmd

P = 128                # SBUF partitions
NCORES = 8             # NeuronCores
SEQ = 100              # tokens per phrase
BPC = 1024             # phrases per core
NGRP = 8               # gpsimd 16-partition groups
BPG = BPC // NGRP      # phrases per group = 128
NI = BPG * SEQ         # idxs per group = 12800
SEG = 2048             # table entries per partition
NSEG = 16              # vocab segments
VPAD = SEG * NSEG      # padded vocab = 32768
NCHUNK = 4             # pipeline chunks over the j axis
CH = NI // NCHUNK      # idxs per chunk = 3200
CHS = CH // SEQ        # phrases per chunk per group = 32

F32 = mybir.dt.float32
I16 = mybir.dt.int16
I8 = mybir.dt.int8

_cached = None


def _build():
    nc = bacc.Bacc("TRN2", debug=False)
    d_table = nc.declare_dram_parameter("table", [P, SEG], F32, isOutput=False)
    d_idx = nc.declare_dram_parameter("idx", [P, NI // 16], I16, isOutput=False)
    d_mask = nc.declare_dram_parameter("mask", [P, NI], I8, isOutput=False)
    d_ones = nc.declare_dram_parameter("ones8", [P, NGRP], F32, isOutput=False)
    d_brep = nc.declare_dram_parameter("brep", [NGRP, 1], F32, isOutput=False)
    d_out = nc.declare_dram_parameter("out", [NGRP, BPG], F32, isOutput=True)

    with tile.TileContext(nc) as tc:
        with (
            tc.tile_pool(name="const", bufs=1) as cpool,
            tc.tile_pool(name="g", bufs=2) as gpool,
            tc.tile_pool(name="p", bufs=2) as ppool,
            tc.tile_pool(name="psum", bufs=1, space="PSUM") as qpool,
        ):
            t_table = cpool.tile([P, SEG], F32)
            t_idx = cpool.tile([P, NI // 16], I16)
            t_mask = cpool.tile([P, NI], I8)
            t_ones = cpool.tile([P, NGRP], F32)
            t_brep = cpool.tile([NGRP, 1], F32)
            t_s = cpool.tile([P, BPG], F32)
            nc.sync.dma_start(out=t_table[:], in_=d_table[:])
            nc.sync.dma_start(out=t_idx[:], in_=d_idx[:])
            nc.sync.dma_start(out=t_mask[:], in_=d_mask[:])
            nc.sync.dma_start(out=t_ones[:], in_=d_ones[:])
            nc.sync.dma_start(out=t_brep[:], in_=d_brep[:])

            for i in range(NCHUNK):
                t_g = gpool.tile([P, CH], F32)
                nc.gpsimd.ap_gather(
                    out_ap=t_g[:],
                    in_ap=t_table[:],
                    idxs_ap=t_idx[:, i * (CH // 16):(i + 1) * (CH // 16)],
                    channels=P, num_elems=SEG, d=1, num_idxs=CH)
                t_p = ppool.tile([P, CH], F32)
                nc.vector.tensor_tensor(
                    out=t_p[:], in0=t_g[:],
                    in1=t_mask[:, i * CH:(i + 1) * CH],
                    op=mybir.AluOpType.mult)
                nc.vector.tensor_reduce(
                    out=t_s[:, i * CHS:(i + 1) * CHS],
                    in_=t_p[:].rearrange("p (b t) -> p b t", t=SEQ),
                    axis=mybir.AxisListType.X,
                    op=mybir.AluOpType.add)

            t_ps = qpool.tile([NGRP, BPG], F32)
            nc.tensor.matmul(out=t_ps[:], lhsT=t_ones[:], rhs=t_s[:],
                             start=True, stop=True)
            t_o = cpool.tile([NGRP, BPG], F32)
            nc.vector.tensor_scalar(
                out=t_o[:], in0=t_ps[:], scalar1=t_brep[:], scalar2=None,
                op0=mybir.AluOpType.add)
            nc.sync.dma_start(out=d_out[:], in_=t_o[:])
    nc.compile()
    return nc


def _prep_inputs(text: np.ndarray, W: np.ndarray, b: np.ndarray):
    """Host-side sharding/layout marshalling (no W-dependent math)."""
    seq, batch = text.shape
    assert seq == SEQ and batch == BPC * NCORES

    wpad = np.zeros(VPAD, np.float32)
    wpad[: W.shape[1]] = W[0].astype(np.float32)
    table = np.tile(wpad.reshape(NSEG, SEG), (NGRP, 1))  # [128, 2048]
    ones8 = np.repeat(np.eye(NGRP, dtype=np.float32), 16, axis=0)  # [128, 8]
    brep = np.full((NGRP, 1), np.float32(b[0]), np.float32)

    in_maps = []
    for c in range(NCORES):
        v = np.ascontiguousarray(text[:, c * BPC:(c + 1) * BPC]).astype(np.int64)
        # j = beta*100 + t per group: [1024, 100] phrase-major, split by group
        vg = v.T.reshape(NGRP, NI)                    # [8, 12800]
        o = (vg & (SEG - 1)).astype(np.int16)         # offsets in segment
        s = (vg >> 11).astype(np.int64)               # segment id 0..15
        # wrapped idx layout: idx[16g + j%16, j//16] = o[g, j]
        idx = o.reshape(NGRP, NI // 16, 16).transpose(0, 2, 1).reshape(P, NI // 16)
        mask = np.zeros((NGRP, NSEG, NI), np.int8)
        mask[np.arange(NGRP)[:, None], s, np.arange(NI)[None, :]] = 1
        in_maps.append({
            "table": table,
            "idx": np.ascontiguousarray(idx),
            "mask": mask.reshape(P, NI),
            "ones8": ones8,
            "brep": brep,
        })
    return in_maps


def kernel(text: np.ndarray, W: np.ndarray, b: np.ndarray) -> np.ndarray:
    global _cached
    if _cached is None:
        _cached = _build()
    nc = _cached
    in_maps = _prep_inputs(np.asarray(text), np.asarray(W), np.asarray(b))
    res = run_bass_kernel_spmd(nc, in_maps, list(range(NCORES)))
    outs = [res.results[c]["out"].reshape(BPC) for c in range(NCORES)]
    return np.concatenate(outs).reshape(BPC * NCORES, 1).astype(np.float32)


if __name__ == "__main__":
    rng = np.random.default_rng(0)
    text = rng.integers(0, 32000, size=(SEQ, BPC * NCORES)).astype(np.int64)
    W = rng.standard_normal((1, 32000)).astype(np.float32)
    b = np.zeros(1, np.float32)
    got = kernel(text, W, b)
    exp = (W[0][text].sum(axis=0) + b[0]).reshape(-1, 1).astype(np.float32)
    err = np.abs(got - exp).max() / np.abs(exp).max()
    print("max abs rel err:", err)
    print("OK" if err < 1e-5 else "FAIL")
